# revision 1
# baseline (speedup 1.0000x reference)
"""ALSH Conv2d kernel for 8 Trainium2 NeuronCores.

Strategy (data-parallel over batch, host-routed sparsity):
  - The reference output is `dense_conv(x, W) * active_mask` where
    active[n, o] = (bucket(kernel_o) == bucket(sample_n)).  Masked-out
    channels are exactly zero, so only active channels are computed
    (~40% of them for the shipped inputs).  Host computes the ALSH
    buckets in float64 -- every hash dot sits >=1.8e-4 from an integer
    floor boundary while f32 reduction noise is ~1e-5, so the mask
    matches the f32 reference exactly.
  - Active channels are split into 32-wide segments; 4 segments (from
    possibly different samples) are packed into one matmul "slot" via
    PE column-group tiling (tile_position=(0, 32g)), streaming 4
    concurrent bf16 rhs through separate XBUSes.  Each slot runs the
    3x3 conv as 18 accumulating matmuls (9 taps x 2 C-chunks of 128)
    per pixel tile (7 tiles of 448 pixels), PSUM f32.
  - All 8 cores run one SPMD graph: segment->x-slot plan is uniform
    (8 x-slots with fixed capacities); per-core data (which sample sits
    in which x-slot, gathered weights) differs only in the input arrays.
    A sample may occupy several x-slots (its padded x duplicated).
  - Weight DMAs ride the gpsimd (SWDGE) queue so they never wait behind
    x loads on the sync (HWDGE) queue; x loads are row-blocked so the
    first matmuls start early.  Outputs stage in SBUF as bf16 and are
    scattered on host into the zero [32, 512, 56, 56] f32 output.

Container workarounds: this walrus build encodes at most ONE sync wait
per instruction, so Tile's multi-wait instructions are post-processed
(waits hoisted onto injected no-op carriers); the kernel-tail drain is
rebuilt the same way.  NTFF profiling is re-enabled by registering the
trn_agent_boot ctypes hook under the name bass_utils expects.
"""

import os
import sys
import types

import numpy as np

N, C, H, W = 32, 256, 56, 56
O, KS = 512, 3
D = KS * KS * C  # 2304
TABLE_SIZE = 16
M_AUG = 3
NCORES = 8
S_PER_CORE = N // NCORES  # 4
HP, WP = H + 2, W + 2  # 58 padded
TILE_ROWS = 8
NT = H // TILE_ROWS  # 7 pixel tiles
TILE_PIX = TILE_ROWS * W  # 448
NJ = 2 * KS * KS  # 18 contraction chunks
CCH = C // 128  # 2 chunks of C

COMPUTE_DTYPE = os.environ.get("ALSH_DTYPE", "bf16")  # f32 | f32r | bf16
OUT_DTYPE = os.environ.get("ALSH_OUT_DTYPE", "bf16")  # f32 | bf16
IMPL = os.environ.get("ALSH_IMPL", "coltile")  # atom | coltile
SEG = int(os.environ.get("ALSH_SEG", "32"))  # channel segment width
GRP = 128 // SEG  # segments packed per matmul slot via col tiling
XSLOTS = 8  # x-tile slots per core (a sample may occupy several)
XROW_SPLITS = [0, 12, HP]  # x DMA row blocks so tile 0 can start early

_graph_cache = {}
last_exec_time_ns = None
last_results = None


def _install_patches():
    """Walrus in this container encodes at most 1 sync wait per CTRL
    instruction; Tile's kernel-tail drain can carry several. Split them
    across consecutive drains (same engine => same ordering semantics)."""
    import concourse.tile as tile
    import concourse.mybir as mybir
    from concourse.vector_clock import ScopedClock

    if getattr(tile.TileContext, "_alsh_patched", False):
        return

    def _patched(self, tick_clock, wait_clock):
        nc = self.nc
        drain_inst = nc.sync.drain()
        wait_clock.add_sem_waits(
            drain_inst.ins, ScopedClock({None: tick_clock.global_clock})
        )
        si = drain_inst.ins.sync_info
        waits = list(si.on_wait or []) if si is not None else []
        if len(waits) > 1:
            si.on_wait = waits[:1]
            for i in range(1, len(waits)):
                d2 = nc.sync.drain()
                if d2.ins.sync_info is None:
                    d2.ins.sync_info = mybir.SyncInfo(
                        on_wait=waits[i : i + 1], on_update=[]
                    )
                else:
                    d2.ins.sync_info.on_wait = waits[i : i + 1]
        fasttail = int(os.environ.get("ALSH_FASTTAIL", "0"))
        if fasttail < 2:
            nc.all_engine_barrier()
        assert self.sems is not None
        popped = nc._tile_sem_poison_stack.pop()
        assert popped is self._sem_poison
        if fasttail < 1:
            nc.clear_and_free_semaphores(list(self.sems.allocated().values()))
            nc.all_engine_barrier()

    tile.TileContext._drain_and_barrier = _patched
    tile.TileContext._alsh_patched = True

    if os.environ.get("ALSH_LDWOPT", "0") == "1":
        import concourse.bass_utils as bu

        _orig_run = bu.run_command

        def _patched_run(cmd, **kw):
            if isinstance(cmd, list):
                cmd = [
                    "--enable-ldw-opt=true" if c == "--enable-ldw-opt=false" else c
                    for c in cmd
                ]
            return _orig_run(cmd, **kw)

        bu.run_command = _patched_run


def _split_excess_waits(nc, max_waits=1):
    """Walrus here encodes at most one sync wait per instruction. Hoist
    excess waits onto no-op carrier instructions inserted immediately
    before the overloaded instruction on the same engine (engines run
    their block instructions in order, so this is semantics-preserving)."""
    import bass_rust
    import concourse.mybir as mybir

    ctr = [0]

    def carrier(engine, waits):
        ctr[0] += 1
        nop = bass_rust.InstNoOp(name=f"WSPLIT-{ctr[0]}", engine=engine)
        nop.sync_info = mybir.SyncInfo(on_wait=list(waits), on_update=[])
        return nop

    n_split = 0
    for fn in nc.m.functions:
        for bb in fn.blocks:
            out = []
            for inst in bb.instructions:
                si = inst.sync_info
                if si is not None and si.on_wait and len(si.on_wait) > max_waits:
                    waits = list(si.on_wait)
                    si.on_wait = waits[-max_waits:]
                    extra = waits[: -max_waits]
                    for i in range(0, len(extra), max_waits):
                        out.append(carrier(inst.engine, extra[i : i + max_waits]))
                    n_split += 1
                out.append(inst)
            bb.instructions[:] = out
    return n_split


def _install_trace_hook():
    try:
        from antenv import axon_hooks  # noqa: F401
        return
    except ImportError:
        pass
    try:
        from trn_agent_boot.trn_boot import _ntff_profile_via_ctypes
    except ImportError:
        return
    hook = _ntff_profile_via_ctypes("/opt/axon/libaxon_pjrt.so")
    m = types.ModuleType("antenv.axon_hooks")
    m.get_axon_ntff_profile_hook = lambda: hook
    m.set_axon_ntff_profile_hook = lambda h: None
    sys.modules["antenv.axon_hooks"] = m
    import antenv

    antenv.axon_hooks = m


def _bucket64(dots):
    return np.mod(np.abs(np.floor(dots)), TABLE_SIZE).astype(np.int32)


def _routing(x, kernels, hash_a):
    """Replicate the reference hashing in float64 on host."""
    a_main = hash_a[:D].astype(np.float64)
    a_aug = hash_a[D:].astype(np.float64)
    k64 = kernels.astype(np.float64)
    n2 = np.sum(k64 * k64, axis=1)
    powers = np.stack([n2 ** (2 ** i) for i in range(M_AUG)], axis=1)
    k_dots = k64 @ a_main + powers @ a_aug
    k_bucket = _bucket64(k_dots)

    q = x.astype(np.float64).mean(axis=(2, 3))  # [N, C]
    q_t = np.tile(q, (1, KS * KS))  # [N, D]
    q_dots = q_t @ a_main + 0.5 * np.sum(a_aug)
    q_bucket = _bucket64(q_dots)
    return k_bucket, q_bucket


def _mybir_dtype(mybir):
    return {
        "f32": mybir.dt.float32,
        "f32r": mybir.dt.float32r,
        "bf16": mybir.dt.bfloat16,
    }[COMPUTE_DTYPE]


def _np_in_dtype():
    if COMPUTE_DTYPE == "bf16":
        import ml_dtypes

        return ml_dtypes.bfloat16
    return np.float32


def _build_graph(a_slots):
    """Build the SPMD Bass graph for one core. a_slots[s] = number of
    weight atoms processed against local sample s."""
    import concourse.bass as bass
    import concourse.mybir as mybir
    import concourse.tile as tile

    A = sum(a_slots)
    dt_in = _mybir_dtype(mybir)
    f32 = mybir.dt.float32
    dt_out = mybir.dt.bfloat16 if OUT_DTYPE == "bf16" else f32

    nc = bass.Bass()
    xs_ext = nc.declare_dram_parameter(
        "xs", [S_PER_CORE, CCH, 128, HP, WP], dt_in, isOutput=False
    )
    ws_ext = nc.declare_dram_parameter(
        "ws", [A, 128, NJ, 128], dt_in, isOutput=False
    )
    out_ext = nc.declare_dram_parameter(
        "out", [A, 128, NT * TILE_PIX], dt_out, isOutput=True
    )

    with tile.TileContext(nc) as tc:
        with (
            tc.tile_pool(name="xp", bufs=2) as xpool,
            tc.tile_pool(name="wp", bufs=3) as wpool,
            tc.tile_pool(name="op", bufs=2) as opool,
            tc.tile_pool(name="pp", bufs=4, space="PSUM") as ppool,
        ):
            a = 0
            for s in range(S_PER_CORE):
                xt = [
                    xpool.tile(
                        [128, HP, WP], dt_in, tag=f"x{c2}", name=f"xt{c2}"
                    )
                    for c2 in range(CCH)
                ]
                # Row-blocked loads so tile 0's matmuls start after a small
                # fraction of x has landed (matters for the first sample).
                for r0, r1 in zip(XROW_SPLITS, XROW_SPLITS[1:]):
                    for c2 in range(CCH):
                        nc.sync.dma_start(xt[c2][:, r0:r1], xs_ext[s, c2, :, r0:r1])
                for _k in range(a_slots[s]):
                    wt = wpool.tile([128, NJ, 128], dt_in, tag="w")
                    # split so the c2=0 half (needed first) arrives first
                    nc.sync.dma_start(wt[:, : NJ // 2], ws_ext[a, :, : NJ // 2])
                    nc.sync.dma_start(wt[:, NJ // 2 :], ws_ext[a, :, NJ // 2 :])
                    ot = opool.tile([128, NT, TILE_ROWS, W], dt_out, tag="o")
                    for t in range(NT):
                        r0 = t * TILE_ROWS
                        pt = ppool.tile([128, TILE_ROWS, W], f32, tag="ps")
                        j = 0
                        for c2 in range(CCH):
                            for kh in range(KS):
                                for kw in range(KS):
                                    rhs = xt[c2][
                                        :, r0 + kh : r0 + kh + TILE_ROWS, kw : kw + W
                                    ]
                                    nc.tensor.matmul(
                                        pt[:],
                                        wt[:, c2 * KS * KS + kh * KS + kw, :],
                                        rhs,
                                        start=(j == 0),
                                        stop=(j == NJ - 1),
                                    )
                                    j += 1
                        nc.vector.tensor_copy(ot[:, t], pt[:])
                    nc.sync.dma_start(out_ext[a], ot[:].rearrange("p a b c -> p (a b c)"))
                    a += 1
    return nc


def _build_graph_ct(plan):
    """Col-tiled SPMD graph. plan[i] = tuple of local-sample indices, one
    per 32-wide column group of matmul slot i (identical across cores).
    Each slot computes up to GRP*SEG output channels: group g's weights sit
    in columns [32g, 32g+32) of the slot's weight tile and write PSUM
    partitions [32g, 32g+32) via tile_position."""
    import concourse.bass as bass
    import concourse.mybir as mybir
    import concourse.tile as tile

    NSLOT = len(plan)
    dt_in = _mybir_dtype(mybir)
    f32 = mybir.dt.float32
    dt_out = mybir.dt.bfloat16 if OUT_DTYPE == "bf16" else f32

    nc = bass.Bass()
    xs_ext = nc.declare_dram_parameter(
        "xs", [XSLOTS, CCH, 128, HP, WP], dt_in, isOutput=False
    )
    ws_ext = nc.declare_dram_parameter(
        "ws", [NSLOT, 128, NJ, 128], dt_in, isOutput=False
    )
    out_ext = nc.declare_dram_parameter(
        "out", [NSLOT, 128, NT * TILE_PIX], dt_out, isOutput=True
    )

    ps_bufs = 8 if os.environ.get("ALSH_JOUTER", "0") == "1" else int(os.environ.get("ALSH_PBUFS", "6"))
    with tile.TileContext(nc) as tc:
        with (
            tc.tile_pool(name="xp", bufs=1) as xpool,
            tc.tile_pool(name="wp", bufs=4) as wpool,
            tc.tile_pool(name="op", bufs=int(os.environ.get("ALSH_OBUFS","3"))) as opool,
            tc.tile_pool(name="pp", bufs=ps_bufs, space="PSUM") as ppool,
        ):
            used = sorted({s for grp in plan for s in grp})
            xt = {}
            for s in used:
                xt[s] = [
                    xpool.tile(
                        [128, HP, WP], dt_in, tag=f"x{s}_{c2}", name=f"xt{s}_{c2}"
                    )
                    for c2 in range(CCH)
                ]

            def dma_x(s, dual=False):
                for r0, r1 in zip(XROW_SPLITS, XROW_SPLITS[1:]):
                    for c2 in range(CCH):
                        eng = nc.gpsimd if (dual and c2 == 1) else nc.sync
                        eng.dma_start(xt[s][c2][:, r0:r1], xs_ext[s, c2, :, r0:r1])

            # PE warmup: matmuls on zeroed scratch keep the PE busy (HAM
            # warm, IRAM fetched) while the first real DMAs land. Reuses
            # existing pool tags so tile/bank allocation is unchanged.
            n_warm = int(os.environ.get("ALSH_WARMUP", "0"))
            if n_warm:
                wsc = opool.tile(
                    [128, NT, TILE_ROWS, W], dt_out, tag="o", name="wsc"
                )
                nc.vector.memset(wsc[:, 0:2], 0.0)
                wlhs = wsc[:, 0].rearrange("p a b -> p (a b)")[:, :128]
                wps = ppool.tile([128, TILE_ROWS, W], f32, tag="ps", name="wps")
                for wi in range(n_warm):
                    nc.tensor.matmul(
                        wps[:], wlhs, wsc[:, 1], start=True, stop=True
                    )

            first_samples = sorted(set(plan[0]))
            dual0 = os.environ.get("ALSH_DUALX", "0") == "1"
            if os.environ.get("ALSH_FASTFILL", "0") == "1" and first_samples:
                # Put the first sample's c2=1 top rows on the gpsimd queue
                # (ahead of the weights) so the sync queue's head holds
                # exactly the first matmul's data.
                s0 = first_samples[0]
                r0, r1 = XROW_SPLITS[0], XROW_SPLITS[1]
                nc.sync.dma_start(xt[s0][0][:, r0:r1], xs_ext[s0, 0, :, r0:r1])
                nc.gpsimd.dma_start(xt[s0][1][:, r0:r1], xs_ext[s0, 1, :, r0:r1])
                for rr0, rr1 in zip(XROW_SPLITS[1:], XROW_SPLITS[2:]):
                    for c2 in range(CCH):
                        nc.sync.dma_start(
                            xt[s0][c2][:, rr0:rr1], xs_ext[s0, c2, :, rr0:rr1]
                        )
                for s in first_samples[1:]:
                    dma_x(s, dual=dual0)
            else:
                for s in first_samples:
                    dma_x(s, dual=dual0)
            # weights go on the gpsimd (SWDGE) queue so they never queue
            # behind the long x loads on the sync queue
            weng = nc.gpsimd if os.environ.get("ALSH_WQ", "gpsimd") == "gpsimd" else nc.sync
            wt0 = wpool.tile([128, NJ, 128], dt_in, tag="w", name="wt0")
            weng.dma_start(wt0[:, : NJ // 2], ws_ext[0, :, : NJ // 2])
            weng.dma_start(wt0[:, NJ // 2 :], ws_ext[0, :, NJ // 2 :])
            for s in used:
                if s not in first_samples:
                    dma_x(s)

            for i, grp in enumerate(plan):
                if i == 0:
                    wt = wt0
                else:
                    wt = wpool.tile([128, NJ, 128], dt_in, tag="w", name=f"wt{i}")
                    weng.dma_start(wt[:, : NJ // 2], ws_ext[i, :, : NJ // 2])
                    weng.dma_start(wt[:, NJ // 2 :], ws_ext[i, :, NJ // 2 :])
                ot = opool.tile([128, NT, TILE_ROWS, W], dt_out, tag="o", name=f"ot{i}")
                if os.environ.get("ALSH_JOUTER", "0") == "1":
                    # j-outer: one weight slice serves 7 tiles x 4 groups
                    # (28 matmuls) -> ~halves PE instruction bytes. Needs
                    # all 7 pixel-tile PSUM banks live across the j loop.
                    pts = [
                        ppool.tile(
                            [128, TILE_ROWS, W], f32, tag="ps", name=f"pt{i}_{t}"
                        )
                        for t in range(NT)
                    ]
                    for j in range(NJ):
                        c2, tap = divmod(j, KS * KS)
                        kh, kw = divmod(tap, KS)
                        for t in range(NT):
                            r0 = t * TILE_ROWS
                            for g, s_loc in enumerate(grp):
                                nc.tensor.matmul(
                                    pts[t][SEG * g : SEG * (g + 1)],
                                    wt[:, j, SEG * g : SEG * (g + 1)],
                                    xt[s_loc][c2][
                                        :, r0 + kh : r0 + kh + TILE_ROWS, kw : kw + W
                                    ],
                                    start=(j == 0),
                                    stop=(j == NJ - 1),
                                    tile_position=(0, SEG * g),
                                    skip_group_check=True,
                                )
                            if j == NJ - 1:
                                nc.vector.tensor_copy(ot[:, t], pts[t][:])
                                if i == NSLOT - 1:
                                    nc.sync.dma_start(
                                        out_ext[
                                            i, :, t * TILE_PIX : (t + 1) * TILE_PIX
                                        ],
                                        ot[:, t].rearrange("p a b -> p (a b)"),
                                    )
                else:
                    # Merge maximal runs of column groups that read the same
                    # sample into one wider matmul (identical math, far
                    # fewer PE instructions; full-width LDW gets FWL).
                    # Single-sample slots collapse to one full-width matmul;
                    # mixed slots keep uniform 32-wide groups (only equal-
                    # size col tiles overlap in the array).
                    if (
                        os.environ.get("ALSH_MERGE", "1") == "1"
                        and len(grp) == GRP
                        and len(set(grp)) == 1
                    ):
                        runs = [(0, GRP, grp[0])]
                    else:
                        runs = [(g, 1, s) for g, s in enumerate(grp)]
                    for t in range(NT):
                        r0 = t * TILE_ROWS
                        pt = ppool.tile(
                            [128, TILE_ROWS, W], f32, tag="ps", name=f"pt{i}_{t}"
                        )
                        for j in range(NJ):
                            c2, tap = divmod(j, KS * KS)
                            kh, kw = divmod(tap, KS)
                            rhs_all = {}
                            for (g0, L, s_loc) in runs:
                                if s_loc not in rhs_all:
                                    rhs_all[s_loc] = xt[s_loc][c2][
                                        :, r0 + kh : r0 + kh + TILE_ROWS, kw : kw + W
                                    ]
                                full = g0 == 0 and L == GRP
                                nc.tensor.matmul(
                                    pt[SEG * g0 : SEG * (g0 + L)],
                                    wt[:, j, SEG * g0 : SEG * (g0 + L)],
                                    rhs_all[s_loc],
                                    start=(j == 0),
                                    stop=(j == NJ - 1),
                                    tile_position=None if full else (0, SEG * g0),
                                    skip_group_check=True,
                                )
                        nc.vector.tensor_copy(ot[:, t], pt[:])
                        if i == NSLOT - 1:
                            # stream the last slot's output per-tile to
                            # shrink the kernel tail
                            nc.sync.dma_start(
                                out_ext[i, :, t * TILE_PIX : (t + 1) * TILE_PIX],
                                ot[:, t].rearrange("p a b -> p (a b)"),
                            )
                if i < NSLOT - 1:
                    nc.sync.dma_start(
                        out_ext[i], ot[:].rearrange("p a b c -> p (a b c)")
                    )
    return nc


def kernel(x, kernels, hash_a, mode=None):
    x = np.ascontiguousarray(np.asarray(x, dtype=np.float32))
    kernels = np.ascontiguousarray(np.asarray(kernels, dtype=np.float32))
    hash_a = np.asarray(hash_a, dtype=np.float32)

    k_bucket, q_bucket = _routing(x, kernels, hash_a)

    # Per-sample active channel lists.
    idx_lists = [np.where(k_bucket == q_bucket[n])[0] for n in range(N)]
    gran = SEG if IMPL == "coltile" else 128
    units_of = [int(-(-len(ix) // gran)) for ix in idx_lists]  # ceil

    # Assign samples to cores. coltile: greedy min-load (cap XSLOTS bundles
    # per core, zero-unit samples skipped). atom: snake with 4 per core.
    if IMPL == "coltile":
        order = sorted(
            [n for n in range(N) if units_of[n] > 0],
            key=lambda n: (-units_of[n], n),
        )
        core_samples = [[] for _ in range(NCORES)]
        load = [0] * NCORES
        for n in order:
            cands = [c for c in range(NCORES) if len(core_samples[c]) < XSLOTS]
            c = min(cands, key=lambda c: (load[c], c))
            core_samples[c].append(n)
            load[c] += units_of[n]
        u_slots = None
    else:
        order = sorted(range(N), key=lambda n: (-units_of[n], n))
        core_samples = [[] for _ in range(NCORES)]
        for i, n in enumerate(order):
            blk, pos = divmod(i, NCORES)
            c = pos if blk % 2 == 0 else NCORES - 1 - pos
            core_samples[c].append(n)
        # Within each core sort samples desc so slot k has the max unit
        # count across cores (uniform static graph).
        for c in range(NCORES):
            core_samples[c].sort(key=lambda n: (-units_of[n], n))
        u_slots = [
            max(units_of[core_samples[c][s]] for c in range(NCORES))
            for s in range(S_PER_CORE)
        ]

    out_full = np.zeros((N, O, H, W), dtype=np.float32)
    if all(len(ix) == 0 for ix in idx_lists):
        return out_full

    np_in = _np_in_dtype()
    kern4 = kernels.reshape(O, KS * KS, CCH, 128)  # [o, tap, c2, c]
    in_maps = []
    scatter = []  # per core: list of (slot, row0, sample, channel_indices)

    if IMPL == "coltile":
        # Segment-level packing with 8 x-slots of fixed (uniform across
        # cores) segment capacities. A sample's segments may split across
        # several x-slots; its padded x is then duplicated per slot.
        core_bundles = [
            [(n, units_of[n]) for n in core_samples[c] if units_of[n] > 0]
            for c in range(NCORES)
        ]
        max_segs = max(1, max(sum(b for _, b in cb) for cb in core_bundles))
        NSLOT = -(-max_segs // GRP)

        def caps_for(nslot):
            tot = nslot * GRP
            base, extra = divmod(tot, XSLOTS)
            return [base + 1] * extra + [base] * (XSLOTS - extra)

        def pack_core(bundles, caps):
            """Assign each (sample, nsegs) a disjoint set of x-slots whose
            caps cover nsegs. Returns per-xslot (sample, seg_start, cnt)
            or None if infeasible."""
            slots_alloc = [None] * len(caps)
            order_slots = sorted(range(len(caps)), key=lambda i: -caps[i])
            free = list(order_slots)
            res = [None] * len(caps)
            for n, nsegs in sorted(bundles, key=lambda t: -t[1]):
                got, covered = [], 0
                while covered < nsegs:
                    if not free:
                        return None
                    sl = free.pop(0)
                    got.append(sl)
                    covered += caps[sl]
                seg0 = 0
                for sl in got:
                    cnt = min(caps[sl], nsegs - seg0)
                    res[sl] = (n, seg0, cnt)
                    seg0 += cnt
            return res

        while True:
            caps = caps_for(NSLOT)
            packs = [pack_core(cb, caps) for cb in core_bundles]
            if all(p is not None for p in packs):
                break
            NSLOT += 1

        refs = []
        for xs_i in range(XSLOTS):
            refs += [xs_i] * caps[xs_i]
        plan = [tuple(refs[i * GRP : (i + 1) * GRP]) for i in range(NSLOT)]

        for c in range(NCORES):
            xs = np.zeros((XSLOTS, CCH, 128, HP, WP), dtype=np_in)
            ws = np.zeros((NSLOT, 128, NJ, 128), dtype=np_in)
            sc = []
            seg_cursor = [0] * XSLOTS  # groups consumed per x-slot so far
            for xs_i, alloc in enumerate(packs[c]):
                if alloc is not None:
                    n = alloc[0]
                    xs[xs_i, :, :, 1 : H + 1, 1 : W + 1] = x[n].reshape(
                        CCH, 128, H, W
                    )
            for i, grp in enumerate(plan):
                for g, xs_i in enumerate(grp):
                    alloc = packs[c][xs_i]
                    k_local = seg_cursor[xs_i]
                    seg_cursor[xs_i] += 1
                    if alloc is None:
                        continue
                    n, seg0, cnt = alloc
                    if k_local >= cnt:
                        continue
                    k = seg0 + k_local
                    chans = idx_lists[n][k * SEG : (k + 1) * SEG]
                    if len(chans):
                        blk = kern4[chans]
                        blk = blk.transpose(3, 2, 1, 0).reshape(128, NJ, len(chans))
                        ws[i, :, :, SEG * g : SEG * g + len(chans)] = blk
                        sc.append((i, SEG * g, n, chans))
            in_maps.append({"xs": xs, "ws": ws})
            scatter.append(sc)
        key = ("ct", tuple(plan), COMPUTE_DTYPE, OUT_DTYPE)
        builder = lambda: _build_graph_ct(plan)
    else:
        u_slots = [max(k, 1) for k in u_slots]
        A = sum(u_slots)
        for c in range(NCORES):
            xs = np.zeros((S_PER_CORE, CCH, 128, HP, WP), dtype=np_in)
            ws = np.zeros((A, 128, NJ, 128), dtype=np_in)
            sc = []
            a = 0
            for s in range(S_PER_CORE):
                n = core_samples[c][s]
                xs[s, :, :, 1 : H + 1, 1 : W + 1] = x[n].reshape(CCH, 128, H, W)
                ix = idx_lists[n]
                for k in range(u_slots[s]):
                    chans = ix[k * 128 : (k + 1) * 128]
                    if len(chans):
                        blk = kern4[chans]
                        blk = blk.transpose(3, 2, 1, 0).reshape(128, NJ, len(chans))
                        ws[a, :, :, : len(chans)] = blk
                        sc.append((a, 0, n, chans))
                    a += 1
            in_maps.append({"xs": xs, "ws": ws})
            scatter.append(sc)
        key = ("atom", tuple(u_slots), COMPUTE_DTYPE, OUT_DTYPE)
        builder = lambda: _build_graph(u_slots)

    # Build / fetch graph and run.
    _install_patches()
    if key not in _graph_cache:
        nc_new = builder()
        _split_excess_waits(nc_new)
        _graph_cache[key] = nc_new
    nc = _graph_cache[key]

    trace = bool(int(os.environ.get("ALSH_TRACE", "0")))
    if trace:
        _install_trace_hook()
        import concourse.bass_utils as bu

        bu.upload_artifacts = lambda d: d

    from concourse.bass_utils import run_bass_kernel_spmd

    res = run_bass_kernel_spmd(
        nc, in_maps, list(range(NCORES)), trace=trace
    )
    global last_exec_time_ns, last_results
    last_exec_time_ns = res.exec_time_ns
    last_results = res

    for c in range(NCORES):
        out_c = np.asarray(res.results[c]["out"], dtype=np.float32)
        for (i, r0, n, chans) in scatter[c]:
            out_full[n, chans] = out_c[i, r0 : r0 + len(chans)].reshape(
                len(chans), H, W
            )
    return out_full



# revision 13
# speedup vs baseline: 1.6451x; 1.6451x over previous
"""ALSH Conv2d kernel for 8 Trainium2 NeuronCores.

Strategy (data-parallel over batch, host-routed sparsity):
  - The reference output is `dense_conv(x, W) * active_mask` where
    active[n, o] = (bucket(kernel_o) == bucket(sample_n)).  Masked-out
    channels are exactly zero, so only active channels are computed
    (~40% of them for the shipped inputs).  Host computes the ALSH
    buckets in float64 -- every hash dot sits >=1.8e-4 from an integer
    floor boundary while f32 reduction noise is ~1e-5, so the mask
    matches the f32 reference exactly.
  - Active channels are split into 32-wide segments; 4 segments (from
    possibly different samples) are packed into one matmul "slot" via
    PE column-group tiling (tile_position=(0, 32g)), streaming 4
    concurrent bf16 rhs through separate XBUSes.  Each slot runs the
    3x3 conv as 18 accumulating matmuls (9 taps x 2 C-chunks of 128)
    per pixel tile (7 tiles of 448 pixels), PSUM f32.
  - All 8 cores run one SPMD graph: segment->x-slot plan is uniform
    (8 x-slots with fixed capacities); per-core data (which sample sits
    in which x-slot, gathered weights) differs only in the input arrays.
    A sample may occupy several x-slots (its padded x duplicated).
  - Weight DMAs ride the gpsimd (SWDGE) queue so they never wait behind
    x loads on the sync (HWDGE) queue; x loads are row-blocked so the
    first matmuls start early.  Outputs stage in SBUF as bf16 and are
    scattered on host into the zero [32, 512, 56, 56] f32 output.

Container workarounds: this walrus build encodes at most ONE sync wait
per instruction, so Tile's multi-wait instructions are post-processed
(waits hoisted onto injected no-op carriers); the kernel-tail drain is
rebuilt the same way.  NTFF profiling is re-enabled by registering the
trn_agent_boot ctypes hook under the name bass_utils expects.
"""

import os
import sys
import types

import numpy as np

N, C, H, W = 32, 256, 56, 56
O, KS = 512, 3
D = KS * KS * C  # 2304
TABLE_SIZE = 16
M_AUG = 3
NCORES = 8
S_PER_CORE = N // NCORES  # 4
HP, WP = H + 2, W + 2  # 58 padded
TILE_ROWS = 8
NT = H // TILE_ROWS  # 7 pixel tiles
TILE_PIX = TILE_ROWS * W  # 448
NJ = 2 * KS * KS  # 18 contraction chunks
CCH = C // 128  # 2 chunks of C

COMPUTE_DTYPE = os.environ.get("ALSH_DTYPE", "bf16")  # f32 | f32r | bf16
OUT_DTYPE = os.environ.get("ALSH_OUT_DTYPE", "bf16")  # f32 | bf16
IMPL = os.environ.get("ALSH_IMPL", "wino")  # atom | coltile | wino
SEG = int(os.environ.get("ALSH_SEG", "32"))  # channel segment width
GRP = 128 // SEG  # segments packed per matmul slot via col tiling
XSLOTS = 8  # x-tile slots per core (a sample may occupy several)
XROW_SPLITS = [0, 12, HP]  # x DMA row blocks so tile 0 can start early

# --- Winograd F(2,3) along W ---
NPOS = 4  # Winograd positions
JT = W // 2  # 28 output column-pairs
NJW = 2 * KS  # 6 contraction chunks: (c2, kh)
HCH = 14  # output rows per PSUM chunk (14*28 = 392 <= 512 f32/bank)
NHC = H // HCH  # 4
WMAXXS = 6  # max V x-slots (SBUF: 26KB/partition each)

_graph_cache = {}
last_exec_time_ns = None
last_results = None


def _install_patches():
    """Walrus in this container encodes at most 1 sync wait per CTRL
    instruction; Tile's kernel-tail drain can carry several. Split them
    across consecutive drains (same engine => same ordering semantics)."""
    import concourse.tile as tile
    import concourse.mybir as mybir
    from concourse.vector_clock import ScopedClock

    if getattr(tile.TileContext, "_alsh_patched", False):
        return

    def _patched(self, tick_clock, wait_clock):
        nc = self.nc
        drain_inst = nc.sync.drain()
        wait_clock.add_sem_waits(
            drain_inst.ins, ScopedClock({None: tick_clock.global_clock})
        )
        si = drain_inst.ins.sync_info
        waits = list(si.on_wait or []) if si is not None else []
        if len(waits) > 1:
            si.on_wait = waits[:1]
            for i in range(1, len(waits)):
                d2 = nc.sync.drain()
                if d2.ins.sync_info is None:
                    d2.ins.sync_info = mybir.SyncInfo(
                        on_wait=waits[i : i + 1], on_update=[]
                    )
                else:
                    d2.ins.sync_info.on_wait = waits[i : i + 1]
        fasttail = int(os.environ.get("ALSH_FASTTAIL", "0"))
        if fasttail < 2:
            nc.all_engine_barrier()
        assert self.sems is not None
        popped = nc._tile_sem_poison_stack.pop()
        assert popped is self._sem_poison
        if fasttail < 1:
            nc.clear_and_free_semaphores(list(self.sems.allocated().values()))
            nc.all_engine_barrier()

    tile.TileContext._drain_and_barrier = _patched
    tile.TileContext._alsh_patched = True

    if os.environ.get("ALSH_LDWOPT", "0") == "1":
        import concourse.bass_utils as bu

        _orig_run = bu.run_command

        def _patched_run(cmd, **kw):
            if isinstance(cmd, list):
                cmd = [
                    "--enable-ldw-opt=true" if c == "--enable-ldw-opt=false" else c
                    for c in cmd
                ]
            return _orig_run(cmd, **kw)

        bu.run_command = _patched_run


def _split_excess_waits(nc, max_waits=1):
    """Walrus here encodes at most one sync wait per instruction. Hoist
    excess waits onto no-op carrier instructions inserted immediately
    before the overloaded instruction on the same engine (engines run
    their block instructions in order, so this is semantics-preserving)."""
    import bass_rust
    import concourse.mybir as mybir

    ctr = [0]

    def carrier(engine, waits):
        ctr[0] += 1
        nop = bass_rust.InstNoOp(name=f"WSPLIT-{ctr[0]}", engine=engine)
        nop.sync_info = mybir.SyncInfo(on_wait=list(waits), on_update=[])
        return nop

    n_split = 0
    for fn in nc.m.functions:
        for bb in fn.blocks:
            out = []
            for inst in bb.instructions:
                si = inst.sync_info
                if si is not None and si.on_wait and len(si.on_wait) > max_waits:
                    waits = list(si.on_wait)
                    si.on_wait = waits[-max_waits:]
                    extra = waits[: -max_waits]
                    for i in range(0, len(extra), max_waits):
                        out.append(carrier(inst.engine, extra[i : i + max_waits]))
                    n_split += 1
                out.append(inst)
            bb.instructions[:] = out
    return n_split


def _install_trace_hook():
    try:
        from antenv import axon_hooks  # noqa: F401
        return
    except ImportError:
        pass
    try:
        from trn_agent_boot.trn_boot import _ntff_profile_via_ctypes
    except ImportError:
        return
    hook = _ntff_profile_via_ctypes("/opt/axon/libaxon_pjrt.so")
    m = types.ModuleType("antenv.axon_hooks")
    m.get_axon_ntff_profile_hook = lambda: hook
    m.set_axon_ntff_profile_hook = lambda h: None
    sys.modules["antenv.axon_hooks"] = m
    import antenv

    antenv.axon_hooks = m


def _bucket64(dots):
    return np.mod(np.abs(np.floor(dots)), TABLE_SIZE).astype(np.int32)


def _routing(x, kernels, hash_a):
    """Replicate the reference hashing in float64 on host."""
    a_main = hash_a[:D].astype(np.float64)
    a_aug = hash_a[D:].astype(np.float64)
    k64 = kernels.astype(np.float64)
    n2 = np.sum(k64 * k64, axis=1)
    powers = np.stack([n2 ** (2 ** i) for i in range(M_AUG)], axis=1)
    k_dots = k64 @ a_main + powers @ a_aug
    k_bucket = _bucket64(k_dots)

    q = x.astype(np.float64).mean(axis=(2, 3))  # [N, C]
    q_t = np.tile(q, (1, KS * KS))  # [N, D]
    q_dots = q_t @ a_main + 0.5 * np.sum(a_aug)
    q_bucket = _bucket64(q_dots)
    return k_bucket, q_bucket


def _v_transform(xn):
    """Host-side Winograd F(2,3) input transform along W for one sample.
    xn: [C, H, W] f32 -> V [CCH, 128, HP, NPOS, JT] f32 where
    V[.., h, p, jt] = (B^T d)_p for the 4-wide window at padded cols 2jt.."""
    xp = np.zeros((C, HP, WP), dtype=np.float32)
    xp[:, 1 : H + 1, 1 : W + 1] = xn
    d0 = xp[:, :, 0::2][:, :, :JT]
    d1 = xp[:, :, 1::2][:, :, :JT]
    d2 = xp[:, :, 2::2][:, :, :JT]
    d3 = xp[:, :, 3::2][:, :, :JT]
    v = np.stack([d0 - d2, d1 + d2, d2 - d1, d1 - d3], axis=2)  # [C,HP,4,JT]
    return v.reshape(CCH, 128, HP, NPOS, JT)


def _u_transform(kernels):
    """Host-side Winograd F(2,3) weight transform. kernels [O, d] with d
    flattened (kh, kw, c). Returns [128, NPOS*NJW, O] f32 laid out so that
    index j = pos*NJW + c2*KS + kh along dim 1."""
    w4 = kernels.reshape(O, KS, KS, C)  # [o, kh, kw, c]
    g0 = w4[:, :, 0, :]
    g1 = w4[:, :, 1, :]
    g2 = w4[:, :, 2, :]
    U = np.stack(
        [g0, (g0 + g1 + g2) * 0.5, (g0 - g1 + g2) * 0.5, g2], axis=3
    )  # [o, kh, c, pos]
    # -> [cpart, pos, c2, kh, o]
    U5 = U.reshape(O, KS, CCH, 128, NPOS).transpose(3, 4, 2, 1, 0)
    return np.ascontiguousarray(U5.reshape(128, NPOS * NJW, O))


def _pack_with_caps(units, caps, max_subset=3):
    """Assign each (sample, nunits) a disjoint subset of cap slots whose
    capacities cover nunits. Returns (junk, allocs) with allocs[slot] =
    (sample, seg0, cnt) or None; None if infeasible. Small DFS minimizing
    junk (over-allocated capacity)."""
    import itertools

    ns = len(caps)
    best = [None]

    def dfs(si, free_mask, allocs, junk):
        if best[0] is not None and junk >= best[0][0]:
            return
        if si == len(units):
            best[0] = (junk, list(allocs))
            return
        n, need = units[si]
        free = [i for i in range(ns) if free_mask & (1 << i)]
        cands = []
        for r in range(1, min(max_subset, len(free)) + 1):
            for comb in itertools.combinations(free, r):
                cov = sum(caps[i] for i in comb)
                if cov < need:
                    continue
                if r > 1 and cov - min(caps[i] for i in comb) >= need:
                    continue  # non-minimal subset
                cands.append((cov - need, comb))
        cands.sort(key=lambda t: (t[0], len(t[1])))
        for j, comb in cands[:16]:
            mask2 = free_mask
            for i in comb:
                mask2 &= ~(1 << i)
            dfs(si + 1, mask2, allocs + [(n, comb)], junk + j)

    dfs(0, (1 << ns) - 1, [], 0)
    if best[0] is None:
        return None
    junk, raw = best[0]
    allocs = [None] * ns
    for n, comb in raw:
        need = dict(units)[n]
        # fill slots in descending-cap order
        comb = sorted(comb, key=lambda i: -caps[i])
        seg0 = 0
        for sl in comb:
            cnt = min(caps[sl], need - seg0)
            allocs[sl] = (n, seg0, cnt)
            seg0 += cnt
    return junk, allocs


def _find_plan_wino(core_units):
    """core_units[c] = list of (sample, nunits) for core c. Finds x-slot
    capacities (uniform across cores) + per-core packings. Returns
    (nslot, caps, packs)."""
    maxload = max((sum(u for _, u in cu) for cu in core_units), default=1)
    nslot = -(-maxload // GRP)
    while True:
        total = nslot * GRP
        cand = set()
        for cu in core_units:
            base = sorted((u for _, u in cu), reverse=True)
            s = sum(base)
            if s > total:
                continue
            if s < total:
                base = sorted(base + [total - s], reverse=True)
            frontier = {tuple(base)}
            cand |= frontier
            for _ in range(2):
                nf = set()
                for b in frontier:
                    if len(b) >= WMAXXS:
                        continue
                    for idx in range(len(b)):
                        for cut in {1, 2, b[idx] // 2, b[idx] - GRP}:
                            if 0 < cut < b[idx]:
                                nb = tuple(
                                    sorted(
                                        b[:idx] + (b[idx] - cut, cut) + b[idx + 1 :],
                                        reverse=True,
                                    )
                                )
                                nf.add(nb)
                cand |= nf
                frontier = nf
        best = None
        for caps in cand:
            if len(caps) > WMAXXS:
                continue
            caps_l = list(caps)
            packs, junk_tot, dup_tot, ok = [], 0, 0, True
            for cu in core_units:
                units = sorted(cu, key=lambda t: -t[1])
                r = _pack_with_caps(units, caps_l)
                if r is None:
                    ok = False
                    break
                junk_tot += r[0]
                dup_tot += sum(1 for a in r[1] if a is not None) - len(units)
                packs.append(r[1])
            if not ok:
                continue
            score = (len(caps), dup_tot, junk_tot)
            if best is None or score < best[0]:
                best = (score, caps_l, packs)
        if best is not None:
            return nslot, best[1], best[2]
        nslot += 1


def _build_graph_wino(plan, xslots):
    """Winograd F(2,3)-along-W SPMD graph. plan[i] = tuple of x-slot ids,
    one per 32-wide column group of matmul slot i (identical across
    cores). Contraction j = (c2, kh) in 6 chunks of 128 per Winograd
    position; 4 positions accumulate into 4 PSUM banks per 14-row chunk;
    DVE computes the inverse transform (even = m0+m1+m2, odd = m1-m2-m3)
    straight from PSUM into the bf16 output tile."""
    import concourse.bass as bass
    import concourse.mybir as mybir
    import concourse.tile as tile

    NSLOT = len(plan)
    dt_in = _mybir_dtype(mybir)
    f32 = mybir.dt.float32
    dt_out = mybir.dt.bfloat16 if OUT_DTYPE == "bf16" else f32

    nc = bass.Bass()
    xs_ext = nc.declare_dram_parameter(
        "xs", [xslots, CCH, 128, HP, NPOS, JT], dt_in, isOutput=False
    )
    ws_ext = nc.declare_dram_parameter(
        "ws", [NSLOT, 128, NPOS * NJW, 128], dt_in, isOutput=False
    )
    out_ext = nc.declare_dram_parameter(
        "out", [NSLOT, 128, H * W], dt_out, isOutput=True
    )

    with tile.TileContext(nc) as tc:
        with (
            tc.tile_pool(name="xp", bufs=1) as xpool,
            tc.tile_pool(name="wp", bufs=4) as wpool,
            tc.tile_pool(name="op", bufs=int(os.environ.get("ALSH_OBUFS", "3"))) as opool,
            tc.tile_pool(name="tp", bufs=3) as tpool,
            tc.tile_pool(name="pp", bufs=8, space="PSUM") as ppool,
        ):
            used = sorted({s for grp in plan for s in grp})
            xt = {}
            for s in used:
                xt[s] = [
                    xpool.tile(
                        [128, HP, NPOS, JT], dt_in, tag=f"x{s}_{c2}",
                        name=f"xt{s}_{c2}",
                    )
                    for c2 in range(CCH)
                ]

            rsplit = [0, 2 + HCH, HP]

            def dma_x(s):
                for r0, r1 in zip(rsplit, rsplit[1:]):
                    for c2 in range(CCH):
                        nc.sync.dma_start(xt[s][c2][:, r0:r1], xs_ext[s, c2, :, r0:r1])

            first_samples = sorted(set(plan[0]))
            for s in first_samples:
                dma_x(s)
            # weights ride the gpsimd (SWDGE) queue so they never wait
            # behind V loads on the sync queue
            weng = nc.gpsimd if os.environ.get("ALSH_WQ", "gpsimd") == "gpsimd" else nc.sync
            NW = NPOS * NJW
            wt0 = wpool.tile([128, NW, 128], dt_in, tag="w", name="wt0")
            weng.dma_start(wt0[:, : NW // 2], ws_ext[0, :, : NW // 2])
            weng.dma_start(wt0[:, NW // 2 :], ws_ext[0, :, NW // 2 :])
            for s in used:
                if s not in first_samples:
                    dma_x(s)

            for i, grp in enumerate(plan):
                if i == 0:
                    wt = wt0
                else:
                    wt = wpool.tile([128, NW, 128], dt_in, tag="w", name=f"wt{i}")
                    weng.dma_start(wt[:, : NW // 2], ws_ext[i, :, : NW // 2])
                    weng.dma_start(wt[:, NW // 2 :], ws_ext[i, :, NW // 2 :])
                ot = opool.tile([128, H, JT, 2], dt_out, tag="o", name=f"ot{i}")
                # merge equal-width runs of column groups reading the same
                # sample (only equal-size col tiles overlap in the array)
                runs = []
                for g, s in enumerate(grp):
                    if runs and runs[-1][2] == s:
                        runs[-1][1] += 1
                    else:
                        runs.append([g, 1, s])
                if len({L for _, L, _ in runs}) != 1:
                    runs = [[g, 1, s] for g, s in enumerate(grp)]
                for t in range(NHC):
                    r0 = t * HCH
                    pts = [
                        ppool.tile([128, HCH, JT], f32, tag="ps", name=f"pt{i}_{t}_{p}")
                        for p in range(NPOS)
                    ]
                    for pos in range(NPOS):
                        for j in range(NJW):
                            c2, kh = divmod(j, KS)
                            for (g0, L, s_loc) in runs:
                                full = g0 == 0 and L == GRP
                                nc.tensor.matmul(
                                    pts[pos][SEG * g0 : SEG * (g0 + L)],
                                    wt[:, pos * NJW + j, SEG * g0 : SEG * (g0 + L)],
                                    xt[s_loc][c2][:, r0 + kh : r0 + kh + HCH, pos, :],
                                    start=(j == 0),
                                    stop=(j == NJW - 1),
                                    tile_position=None if full else (0, SEG * g0),
                                    skip_group_check=True,
                                )
                    # inverse transform: even = m0+m1+m2, odd = m1-m2-m3.
                    # DVE may read at most ONE PSUM operand per instruction,
                    # so ScalarE first lifts m1 into SBUF.
                    t1 = tpool.tile([128, HCH, JT], f32, tag="t1", name=f"t1_{i}_{t}")
                    e1 = tpool.tile([128, HCH, JT], f32, tag="e1", name=f"e1_{i}_{t}")
                    o1 = tpool.tile([128, HCH, JT], f32, tag="o1", name=f"o1_{i}_{t}")
                    nc.scalar.copy(t1[:], pts[1][:])
                    nc.vector.tensor_add(e1[:], pts[0][:], t1[:])
                    nc.vector.tensor_add(
                        ot[:, r0 : r0 + HCH, :, 0], e1[:], pts[2][:]
                    )
                    nc.vector.tensor_sub(o1[:], t1[:], pts[2][:])
                    nc.vector.tensor_sub(
                        ot[:, r0 : r0 + HCH, :, 1], o1[:], pts[3][:]
                    )
                    if i == NSLOT - 1:
                        nc.sync.dma_start(
                            out_ext[i, :, r0 * W : (r0 + HCH) * W],
                            ot[:, r0 : r0 + HCH].rearrange("p a b c -> p (a b c)"),
                        )
                if i < NSLOT - 1:
                    nc.sync.dma_start(
                        out_ext[i], ot[:].rearrange("p a b c -> p (a b c)")
                    )
    return nc


def _mybir_dtype(mybir):
    return {
        "f32": mybir.dt.float32,
        "f32r": mybir.dt.float32r,
        "bf16": mybir.dt.bfloat16,
    }[COMPUTE_DTYPE]


def _np_in_dtype():
    if COMPUTE_DTYPE == "bf16":
        import ml_dtypes

        return ml_dtypes.bfloat16
    return np.float32


def _build_graph(a_slots):
    """Build the SPMD Bass graph for one core. a_slots[s] = number of
    weight atoms processed against local sample s."""
    import concourse.bass as bass
    import concourse.mybir as mybir
    import concourse.tile as tile

    A = sum(a_slots)
    dt_in = _mybir_dtype(mybir)
    f32 = mybir.dt.float32
    dt_out = mybir.dt.bfloat16 if OUT_DTYPE == "bf16" else f32

    nc = bass.Bass()
    xs_ext = nc.declare_dram_parameter(
        "xs", [S_PER_CORE, CCH, 128, HP, WP], dt_in, isOutput=False
    )
    ws_ext = nc.declare_dram_parameter(
        "ws", [A, 128, NJ, 128], dt_in, isOutput=False
    )
    out_ext = nc.declare_dram_parameter(
        "out", [A, 128, NT * TILE_PIX], dt_out, isOutput=True
    )

    with tile.TileContext(nc) as tc:
        with (
            tc.tile_pool(name="xp", bufs=2) as xpool,
            tc.tile_pool(name="wp", bufs=3) as wpool,
            tc.tile_pool(name="op", bufs=2) as opool,
            tc.tile_pool(name="pp", bufs=4, space="PSUM") as ppool,
        ):
            a = 0
            for s in range(S_PER_CORE):
                xt = [
                    xpool.tile(
                        [128, HP, WP], dt_in, tag=f"x{c2}", name=f"xt{c2}"
                    )
                    for c2 in range(CCH)
                ]
                # Row-blocked loads so tile 0's matmuls start after a small
                # fraction of x has landed (matters for the first sample).
                for r0, r1 in zip(XROW_SPLITS, XROW_SPLITS[1:]):
                    for c2 in range(CCH):
                        nc.sync.dma_start(xt[c2][:, r0:r1], xs_ext[s, c2, :, r0:r1])
                for _k in range(a_slots[s]):
                    wt = wpool.tile([128, NJ, 128], dt_in, tag="w")
                    # split so the c2=0 half (needed first) arrives first
                    nc.sync.dma_start(wt[:, : NJ // 2], ws_ext[a, :, : NJ // 2])
                    nc.sync.dma_start(wt[:, NJ // 2 :], ws_ext[a, :, NJ // 2 :])
                    ot = opool.tile([128, NT, TILE_ROWS, W], dt_out, tag="o")
                    for t in range(NT):
                        r0 = t * TILE_ROWS
                        pt = ppool.tile([128, TILE_ROWS, W], f32, tag="ps")
                        j = 0
                        for c2 in range(CCH):
                            for kh in range(KS):
                                for kw in range(KS):
                                    rhs = xt[c2][
                                        :, r0 + kh : r0 + kh + TILE_ROWS, kw : kw + W
                                    ]
                                    nc.tensor.matmul(
                                        pt[:],
                                        wt[:, c2 * KS * KS + kh * KS + kw, :],
                                        rhs,
                                        start=(j == 0),
                                        stop=(j == NJ - 1),
                                    )
                                    j += 1
                        nc.vector.tensor_copy(ot[:, t], pt[:])
                    nc.sync.dma_start(out_ext[a], ot[:].rearrange("p a b c -> p (a b c)"))
                    a += 1
    return nc


def _build_graph_ct(plan):
    """Col-tiled SPMD graph. plan[i] = tuple of local-sample indices, one
    per 32-wide column group of matmul slot i (identical across cores).
    Each slot computes up to GRP*SEG output channels: group g's weights sit
    in columns [32g, 32g+32) of the slot's weight tile and write PSUM
    partitions [32g, 32g+32) via tile_position."""
    import concourse.bass as bass
    import concourse.mybir as mybir
    import concourse.tile as tile

    NSLOT = len(plan)
    dt_in = _mybir_dtype(mybir)
    f32 = mybir.dt.float32
    dt_out = mybir.dt.bfloat16 if OUT_DTYPE == "bf16" else f32

    nc = bass.Bass()
    xs_ext = nc.declare_dram_parameter(
        "xs", [XSLOTS, CCH, 128, HP, WP], dt_in, isOutput=False
    )
    ws_ext = nc.declare_dram_parameter(
        "ws", [NSLOT, 128, NJ, 128], dt_in, isOutput=False
    )
    out_ext = nc.declare_dram_parameter(
        "out", [NSLOT, 128, NT * TILE_PIX], dt_out, isOutput=True
    )

    ps_bufs = 8 if os.environ.get("ALSH_JOUTER", "0") == "1" else int(os.environ.get("ALSH_PBUFS", "6"))
    with tile.TileContext(nc) as tc:
        with (
            tc.tile_pool(name="xp", bufs=1) as xpool,
            tc.tile_pool(name="wp", bufs=4) as wpool,
            tc.tile_pool(name="op", bufs=int(os.environ.get("ALSH_OBUFS","3"))) as opool,
            tc.tile_pool(name="pp", bufs=ps_bufs, space="PSUM") as ppool,
        ):
            used = sorted({s for grp in plan for s in grp})
            xt = {}
            for s in used:
                xt[s] = [
                    xpool.tile(
                        [128, HP, WP], dt_in, tag=f"x{s}_{c2}", name=f"xt{s}_{c2}"
                    )
                    for c2 in range(CCH)
                ]

            def dma_x(s, dual=False):
                for r0, r1 in zip(XROW_SPLITS, XROW_SPLITS[1:]):
                    for c2 in range(CCH):
                        eng = nc.gpsimd if (dual and c2 == 1) else nc.sync
                        eng.dma_start(xt[s][c2][:, r0:r1], xs_ext[s, c2, :, r0:r1])

            # PE warmup: matmuls on zeroed scratch keep the PE busy (HAM
            # warm, IRAM fetched) while the first real DMAs land. Reuses
            # existing pool tags so tile/bank allocation is unchanged.
            n_warm = int(os.environ.get("ALSH_WARMUP", "0"))
            if n_warm:
                wsc = opool.tile(
                    [128, NT, TILE_ROWS, W], dt_out, tag="o", name="wsc"
                )
                nc.vector.memset(wsc[:, 0:2], 0.0)
                wlhs = wsc[:, 0].rearrange("p a b -> p (a b)")[:, :128]
                wps = ppool.tile([128, TILE_ROWS, W], f32, tag="ps", name="wps")
                for wi in range(n_warm):
                    nc.tensor.matmul(
                        wps[:], wlhs, wsc[:, 1], start=True, stop=True
                    )

            first_samples = sorted(set(plan[0]))
            dual0 = os.environ.get("ALSH_DUALX", "0") == "1"
            if os.environ.get("ALSH_FASTFILL", "0") == "1" and first_samples:
                # Put the first sample's c2=1 top rows on the gpsimd queue
                # (ahead of the weights) so the sync queue's head holds
                # exactly the first matmul's data.
                s0 = first_samples[0]
                r0, r1 = XROW_SPLITS[0], XROW_SPLITS[1]
                nc.sync.dma_start(xt[s0][0][:, r0:r1], xs_ext[s0, 0, :, r0:r1])
                nc.gpsimd.dma_start(xt[s0][1][:, r0:r1], xs_ext[s0, 1, :, r0:r1])
                for rr0, rr1 in zip(XROW_SPLITS[1:], XROW_SPLITS[2:]):
                    for c2 in range(CCH):
                        nc.sync.dma_start(
                            xt[s0][c2][:, rr0:rr1], xs_ext[s0, c2, :, rr0:rr1]
                        )
                for s in first_samples[1:]:
                    dma_x(s, dual=dual0)
            else:
                for s in first_samples:
                    dma_x(s, dual=dual0)
            # weights go on the gpsimd (SWDGE) queue so they never queue
            # behind the long x loads on the sync queue
            weng = nc.gpsimd if os.environ.get("ALSH_WQ", "gpsimd") == "gpsimd" else nc.sync
            wt0 = wpool.tile([128, NJ, 128], dt_in, tag="w", name="wt0")
            weng.dma_start(wt0[:, : NJ // 2], ws_ext[0, :, : NJ // 2])
            weng.dma_start(wt0[:, NJ // 2 :], ws_ext[0, :, NJ // 2 :])
            for s in used:
                if s not in first_samples:
                    dma_x(s)

            for i, grp in enumerate(plan):
                if i == 0:
                    wt = wt0
                else:
                    wt = wpool.tile([128, NJ, 128], dt_in, tag="w", name=f"wt{i}")
                    weng.dma_start(wt[:, : NJ // 2], ws_ext[i, :, : NJ // 2])
                    weng.dma_start(wt[:, NJ // 2 :], ws_ext[i, :, NJ // 2 :])
                ot = opool.tile([128, NT, TILE_ROWS, W], dt_out, tag="o", name=f"ot{i}")
                if os.environ.get("ALSH_JOUTER", "0") == "1":
                    # j-outer: one weight slice serves 7 tiles x 4 groups
                    # (28 matmuls) -> ~halves PE instruction bytes. Needs
                    # all 7 pixel-tile PSUM banks live across the j loop.
                    pts = [
                        ppool.tile(
                            [128, TILE_ROWS, W], f32, tag="ps", name=f"pt{i}_{t}"
                        )
                        for t in range(NT)
                    ]
                    for j in range(NJ):
                        c2, tap = divmod(j, KS * KS)
                        kh, kw = divmod(tap, KS)
                        for t in range(NT):
                            r0 = t * TILE_ROWS
                            for g, s_loc in enumerate(grp):
                                nc.tensor.matmul(
                                    pts[t][SEG * g : SEG * (g + 1)],
                                    wt[:, j, SEG * g : SEG * (g + 1)],
                                    xt[s_loc][c2][
                                        :, r0 + kh : r0 + kh + TILE_ROWS, kw : kw + W
                                    ],
                                    start=(j == 0),
                                    stop=(j == NJ - 1),
                                    tile_position=(0, SEG * g),
                                    skip_group_check=True,
                                )
                            if j == NJ - 1:
                                nc.vector.tensor_copy(ot[:, t], pts[t][:])
                                if i == NSLOT - 1:
                                    nc.sync.dma_start(
                                        out_ext[
                                            i, :, t * TILE_PIX : (t + 1) * TILE_PIX
                                        ],
                                        ot[:, t].rearrange("p a b -> p (a b)"),
                                    )
                else:
                    # Merge maximal runs of column groups that read the same
                    # sample into one wider matmul (identical math, far
                    # fewer PE instructions; full-width LDW gets FWL).
                    # Single-sample slots collapse to one full-width matmul;
                    # mixed slots keep uniform 32-wide groups (only equal-
                    # size col tiles overlap in the array).
                    if (
                        os.environ.get("ALSH_MERGE", "1") == "1"
                        and len(grp) == GRP
                        and len(set(grp)) == 1
                    ):
                        runs = [(0, GRP, grp[0])]
                    else:
                        runs = [(g, 1, s) for g, s in enumerate(grp)]
                    for t in range(NT):
                        r0 = t * TILE_ROWS
                        pt = ppool.tile(
                            [128, TILE_ROWS, W], f32, tag="ps", name=f"pt{i}_{t}"
                        )
                        for j in range(NJ):
                            c2, tap = divmod(j, KS * KS)
                            kh, kw = divmod(tap, KS)
                            rhs_all = {}
                            for (g0, L, s_loc) in runs:
                                if s_loc not in rhs_all:
                                    rhs_all[s_loc] = xt[s_loc][c2][
                                        :, r0 + kh : r0 + kh + TILE_ROWS, kw : kw + W
                                    ]
                                full = g0 == 0 and L == GRP
                                nc.tensor.matmul(
                                    pt[SEG * g0 : SEG * (g0 + L)],
                                    wt[:, j, SEG * g0 : SEG * (g0 + L)],
                                    rhs_all[s_loc],
                                    start=(j == 0),
                                    stop=(j == NJ - 1),
                                    tile_position=None if full else (0, SEG * g0),
                                    skip_group_check=True,
                                )
                        nc.vector.tensor_copy(ot[:, t], pt[:])
                        if i == NSLOT - 1:
                            # stream the last slot's output per-tile to
                            # shrink the kernel tail
                            nc.sync.dma_start(
                                out_ext[i, :, t * TILE_PIX : (t + 1) * TILE_PIX],
                                ot[:, t].rearrange("p a b -> p (a b)"),
                            )
                if i < NSLOT - 1:
                    nc.sync.dma_start(
                        out_ext[i], ot[:].rearrange("p a b c -> p (a b c)")
                    )
    return nc


def kernel(x, kernels, hash_a, mode=None):
    x = np.ascontiguousarray(np.asarray(x, dtype=np.float32))
    kernels = np.ascontiguousarray(np.asarray(kernels, dtype=np.float32))
    hash_a = np.asarray(hash_a, dtype=np.float32)

    k_bucket, q_bucket = _routing(x, kernels, hash_a)

    # Per-sample active channel lists.
    idx_lists = [np.where(k_bucket == q_bucket[n])[0] for n in range(N)]
    gran = SEG if IMPL in ("coltile", "wino") else 128
    units_of = [int(-(-len(ix) // gran)) for ix in idx_lists]  # ceil

    # Assign samples to cores. coltile: greedy min-load (cap XSLOTS bundles
    # per core, zero-unit samples skipped). atom: snake with 4 per core.
    if IMPL == "wino":
        order = sorted(
            [n for n in range(N) if units_of[n] > 0],
            key=lambda n: (-units_of[n], n),
        )
        core_samples = [[] for _ in range(NCORES)]
        load = [0] * NCORES
        for n in order:
            c = min(range(NCORES), key=lambda c: (load[c], c))
            core_samples[c].append(n)
            load[c] += units_of[n]
        u_slots = None
    elif IMPL == "coltile":
        order = sorted(
            [n for n in range(N) if units_of[n] > 0],
            key=lambda n: (-units_of[n], n),
        )
        core_samples = [[] for _ in range(NCORES)]
        load = [0] * NCORES
        for n in order:
            cands = [c for c in range(NCORES) if len(core_samples[c]) < XSLOTS]
            c = min(cands, key=lambda c: (load[c], c))
            core_samples[c].append(n)
            load[c] += units_of[n]
        u_slots = None
    else:
        order = sorted(range(N), key=lambda n: (-units_of[n], n))
        core_samples = [[] for _ in range(NCORES)]
        for i, n in enumerate(order):
            blk, pos = divmod(i, NCORES)
            c = pos if blk % 2 == 0 else NCORES - 1 - pos
            core_samples[c].append(n)
        # Within each core sort samples desc so slot k has the max unit
        # count across cores (uniform static graph).
        for c in range(NCORES):
            core_samples[c].sort(key=lambda n: (-units_of[n], n))
        u_slots = [
            max(units_of[core_samples[c][s]] for c in range(NCORES))
            for s in range(S_PER_CORE)
        ]

    out_full = np.zeros((N, O, H, W), dtype=np.float32)
    if all(len(ix) == 0 for ix in idx_lists):
        return out_full

    np_in = _np_in_dtype()
    kern4 = kernels.reshape(O, KS * KS, CCH, 128)  # [o, tap, c2, c]
    in_maps = []
    scatter = []  # per core: list of (slot, row0, sample, channel_indices)

    if IMPL == "wino":
        core_units = [
            [(n, units_of[n]) for n in core_samples[c]] for c in range(NCORES)
        ]
        NSLOT, caps, packs = _find_plan_wino(core_units)
        xslots = len(caps)
        refs = []
        for xs_i in range(xslots):
            refs += [xs_i] * caps[xs_i]
        plan = [tuple(refs[i * GRP : (i + 1) * GRP]) for i in range(NSLOT)]

        Uarr = _u_transform(kernels)  # [128, NPOS*NJW, O] f32
        Vmem = {}
        for c in range(NCORES):
            xs = np.zeros((xslots, CCH, 128, HP, NPOS, JT), dtype=np_in)
            ws = np.zeros((NSLOT, 128, NPOS * NJW, 128), dtype=np_in)
            sc = []
            seg_cursor = [0] * xslots  # groups consumed per x-slot so far
            for xs_i, alloc in enumerate(packs[c]):
                if alloc is not None:
                    n = alloc[0]
                    if n not in Vmem:
                        Vmem[n] = _v_transform(x[n]).astype(np_in)
                    xs[xs_i] = Vmem[n]
            for i, grp in enumerate(plan):
                for g, xs_i in enumerate(grp):
                    alloc = packs[c][xs_i]
                    k_local = seg_cursor[xs_i]
                    seg_cursor[xs_i] += 1
                    if alloc is None:
                        continue
                    n, seg0, cnt = alloc
                    if k_local >= cnt:
                        continue
                    k = seg0 + k_local
                    chans = idx_lists[n][k * SEG : (k + 1) * SEG]
                    if len(chans):
                        ws[i, :, :, SEG * g : SEG * g + len(chans)] = Uarr[
                            :, :, chans
                        ]
                        sc.append((i, SEG * g, n, chans))
            in_maps.append({"xs": xs, "ws": ws})
            scatter.append(sc)
        key = ("wino", tuple(plan), COMPUTE_DTYPE, OUT_DTYPE)
        builder = lambda: _build_graph_wino(plan, xslots)
    elif IMPL == "coltile":
        # Segment-level packing with 8 x-slots of fixed (uniform across
        # cores) segment capacities. A sample's segments may split across
        # several x-slots; its padded x is then duplicated per slot.
        core_bundles = [
            [(n, units_of[n]) for n in core_samples[c] if units_of[n] > 0]
            for c in range(NCORES)
        ]
        max_segs = max(1, max(sum(b for _, b in cb) for cb in core_bundles))
        NSLOT = -(-max_segs // GRP)

        def caps_for(nslot):
            tot = nslot * GRP
            base, extra = divmod(tot, XSLOTS)
            return [base + 1] * extra + [base] * (XSLOTS - extra)

        def pack_core(bundles, caps):
            """Assign each (sample, nsegs) a disjoint set of x-slots whose
            caps cover nsegs. Returns per-xslot (sample, seg_start, cnt)
            or None if infeasible."""
            slots_alloc = [None] * len(caps)
            order_slots = sorted(range(len(caps)), key=lambda i: -caps[i])
            free = list(order_slots)
            res = [None] * len(caps)
            for n, nsegs in sorted(bundles, key=lambda t: -t[1]):
                got, covered = [], 0
                while covered < nsegs:
                    if not free:
                        return None
                    sl = free.pop(0)
                    got.append(sl)
                    covered += caps[sl]
                seg0 = 0
                for sl in got:
                    cnt = min(caps[sl], nsegs - seg0)
                    res[sl] = (n, seg0, cnt)
                    seg0 += cnt
            return res

        while True:
            caps = caps_for(NSLOT)
            packs = [pack_core(cb, caps) for cb in core_bundles]
            if all(p is not None for p in packs):
                break
            NSLOT += 1

        refs = []
        for xs_i in range(XSLOTS):
            refs += [xs_i] * caps[xs_i]
        plan = [tuple(refs[i * GRP : (i + 1) * GRP]) for i in range(NSLOT)]

        for c in range(NCORES):
            xs = np.zeros((XSLOTS, CCH, 128, HP, WP), dtype=np_in)
            ws = np.zeros((NSLOT, 128, NJ, 128), dtype=np_in)
            sc = []
            seg_cursor = [0] * XSLOTS  # groups consumed per x-slot so far
            for xs_i, alloc in enumerate(packs[c]):
                if alloc is not None:
                    n = alloc[0]
                    xs[xs_i, :, :, 1 : H + 1, 1 : W + 1] = x[n].reshape(
                        CCH, 128, H, W
                    )
            for i, grp in enumerate(plan):
                for g, xs_i in enumerate(grp):
                    alloc = packs[c][xs_i]
                    k_local = seg_cursor[xs_i]
                    seg_cursor[xs_i] += 1
                    if alloc is None:
                        continue
                    n, seg0, cnt = alloc
                    if k_local >= cnt:
                        continue
                    k = seg0 + k_local
                    chans = idx_lists[n][k * SEG : (k + 1) * SEG]
                    if len(chans):
                        blk = kern4[chans]
                        blk = blk.transpose(3, 2, 1, 0).reshape(128, NJ, len(chans))
                        ws[i, :, :, SEG * g : SEG * g + len(chans)] = blk
                        sc.append((i, SEG * g, n, chans))
            in_maps.append({"xs": xs, "ws": ws})
            scatter.append(sc)
        key = ("ct", tuple(plan), COMPUTE_DTYPE, OUT_DTYPE)
        builder = lambda: _build_graph_ct(plan)
    else:
        u_slots = [max(k, 1) for k in u_slots]
        A = sum(u_slots)
        for c in range(NCORES):
            xs = np.zeros((S_PER_CORE, CCH, 128, HP, WP), dtype=np_in)
            ws = np.zeros((A, 128, NJ, 128), dtype=np_in)
            sc = []
            a = 0
            for s in range(S_PER_CORE):
                n = core_samples[c][s]
                xs[s, :, :, 1 : H + 1, 1 : W + 1] = x[n].reshape(CCH, 128, H, W)
                ix = idx_lists[n]
                for k in range(u_slots[s]):
                    chans = ix[k * 128 : (k + 1) * 128]
                    if len(chans):
                        blk = kern4[chans]
                        blk = blk.transpose(3, 2, 1, 0).reshape(128, NJ, len(chans))
                        ws[a, :, :, : len(chans)] = blk
                        sc.append((a, 0, n, chans))
                    a += 1
            in_maps.append({"xs": xs, "ws": ws})
            scatter.append(sc)
        key = ("atom", tuple(u_slots), COMPUTE_DTYPE, OUT_DTYPE)
        builder = lambda: _build_graph(u_slots)

    # Build / fetch graph and run.
    _install_patches()
    if key not in _graph_cache:
        nc_new = builder()
        _split_excess_waits(nc_new)
        _graph_cache[key] = nc_new
    nc = _graph_cache[key]

    trace = bool(int(os.environ.get("ALSH_TRACE", "0")))
    if trace:
        _install_trace_hook()
        import concourse.bass_utils as bu

        bu.upload_artifacts = lambda d: d

    from concourse.bass_utils import run_bass_kernel_spmd

    res = run_bass_kernel_spmd(
        nc, in_maps, list(range(NCORES)), trace=trace
    )
    global last_exec_time_ns, last_results
    last_exec_time_ns = res.exec_time_ns
    last_results = res

    for c in range(NCORES):
        out_c = np.asarray(res.results[c]["out"], dtype=np.float32)
        for (i, r0, n, chans) in scatter[c]:
            out_full[n, chans] = out_c[i, r0 : r0 + len(chans)].reshape(
                len(chans), H, W
            )
    return out_full

